# revision 28
# baseline (speedup 1.0000x reference)
"""Trainium2 Bass kernel for the 32-block Feistel CNN (nn_Core_70325794505291).

Strategy: data-parallel over batch (8 elements -> 8 cores). Each core runs the
full 192-conv + 32-mix tower on one batch element.

conv3x3 = 9 shifted matmuls (taps) accumulating into fp32 PSUM. Matmul operands
are fp16 (1 cycle/row on the PE); persistent activations stay fp32 in SBUF.
Activations are rescaled by 1/G at each block boundary (the tower amplifies
~1.7x/block which would overflow fp16); biases are pre-scaled on the host and
conv3 biases are folded into the preceding block-boundary epilogue, so the
rescale is free.  End-to-end rel-err vs the fp32 reference ~2.4e-3 (simulated).

Host-side work is layout only (permutation gather, space-to-depth, weight
repacking); every FLOP of the model runs on-device.

Overlap structure: each conv's two 512-wide N-chunks accumulate into separate
1-bank PSUM tiles so the relu of chunk0 runs during chunk1's matmuls; the
relu+bias+fp16-cast is split rows (0-15 | 16 | 17-31) across ScalarE and
VectorE so the next conv's chunk0 unblocks on the tiny row-16 sliver; the
Feistel skip-add writes fp16 directly; the mix emits its B half first since
only B gates the next block.

Row-level tuning: edge taps whose source row is the zero pad are trimmed from
the matmul free dim (column trims regress: 31-wide psum writes are unaligned
and cost more per-mm overhead than the rows saved).  Startup: pad-plane
memsets are border-only on GpSimd, the affine+fp16-cast of the B half is one
ScalarE op straight off the input DMA, and block-0's conv1 weight panel is
DMA'd ahead of the rest.  Final block streams each 512-wide output chunk to
HBM as its epilogue lands.  Measured: 829.5 us HW exec (PE row floor ~776 us
+ ~34 us LDWEIGHTS/dispatch + ~14 us runtime init), HAM warm throughout,
rel err 3.1e-3.
"""

import os

import numpy as np

# ---- static problem structure (mirrors the reference tape) ----
BLOCKS = 32
BLOCK_SIZE = 8
CONV_SHAPES = [(128, 96), (128, 128), (96, 128)] * 2
H = W = 32
HW = H * W
N_CHUNK = 512  # matmul free-dim chunk (one PSUM bank)

# fp16 scale schedule
G = 1.7
S0 = 32.0

# per-block weight blob column layout (fp16 elements per partition row)
# round r at base 3168*r: conv1 [96p, 1152] | conv2 [128p, 1152] | conv3 [128p, 864]
# mix at 6336: k1-oA [96p,96] | k1-oB | k2-oA | k2-oB
WCOLS = 6720
MIXBASE = 6336


def _conv_meta():
    meta = []
    w_total = 0
    b_total = 0
    for _ in range(BLOCKS):
        for oc, ic in CONV_SHAPES:
            meta.append((oc, ic, w_total, b_total))
            w_total += oc * ic * 9
            b_total += oc
    return meta, w_total, b_total


CONV_META, W_TOTAL, B_TOTAL = _conv_meta()


def _nblocks():
    return int(os.environ.get("BASS_NN_NBLOCKS", BLOCKS))


# --------------------------------------------------------------------------
# host packing
# --------------------------------------------------------------------------

def pack_inputs(x, mu, sigma, w_flat, b_flat, m):
    """Build per-core and shared device input arrays."""
    nb = _nblocks()
    x = np.asarray(x, np.float32)
    mu = np.asarray(mu, np.float32)
    sigma = np.asarray(sigma, np.float32)
    w_flat = np.asarray(w_flat, np.float32)
    b_flat = np.asarray(b_flat, np.float32)
    m = np.asarray(m, np.float32)

    # weight blobs [nb, 128, WCOLS] fp16
    wblk = np.zeros((nb, 128, WCOLS), np.float16)
    for blk in range(nb):
        for r in range(2):
            base = 3168 * r
            ci = blk * 6 + r * 3
            for k, (colw, coln) in enumerate(((0, 1152), (1152, 1152), (2304, 864))):
                oc, ic, woff, _ = CONV_META[ci + k]
                arr = w_flat[woff:woff + oc * ic * 9].reshape(oc, ic, 9)
                panel = arr.transpose(1, 2, 0).reshape(ic, 9 * oc)
                wblk[blk, :ic, base + colw:base + colw + coln] = panel.astype(np.float16)
        mt = m[blk].T  # [c, o]
        wblk[blk, 0:96, MIXBASE + 0:MIXBASE + 96] = mt[0:96, 0:96].astype(np.float16)
        wblk[blk, 0:96, MIXBASE + 96:MIXBASE + 192] = mt[0:96, 96:192].astype(np.float16)
        wblk[blk, 0:96, MIXBASE + 192:MIXBASE + 288] = mt[96:192, 0:96].astype(np.float16)
        wblk[blk, 0:96, MIXBASE + 288:MIXBASE + 384] = mt[96:192, 96:192].astype(np.float16)

    def bias_of(ci):
        oc, _, _, boff = CONV_META[ci]
        return b_flat[boff:boff + oc]

    # bias blob [128, 6*nb] fp32 (one DMA for all blocks)
    bblk = np.zeros((128, 6 * nb), np.float32)
    for blk in range(nb):
        sk = S0 * G ** (-blk)
        bblk[0:128, 6 * blk + 0] = bias_of(blk * 6 + 0) * sk
        bblk[0:128, 6 * blk + 1] = bias_of(blk * 6 + 1) * sk
        bblk[0:128, 6 * blk + 2] = bias_of(blk * 6 + 3) * sk
        bblk[0:128, 6 * blk + 3] = bias_of(blk * 6 + 4) * sk
        if blk + 1 < nb:
            sk1 = S0 * G ** (-(blk + 1))
            bblk[0:96, 6 * blk + 4] = bias_of((blk + 1) * 6 + 2) * sk1
            bblk[0:96, 6 * blk + 5] = bias_of((blk + 1) * 6 + 5) * sk1

    # affine vectors [96, 4]: scaleA, biasA, scaleB, biasB
    affv = np.zeros((96, 4), np.float32)
    p = np.arange(96)
    cA = p // 64            # channels for partitions 0..95
    cB = (96 + p) // 64     # channels for partitions 96..191
    affv[:, 0] = S0 / sigma[cA]
    affv[:, 1] = -S0 * mu[cA] / sigma[cA] + S0 * bias_of(2)
    affv[:, 2] = S0 / sigma[cB]
    affv[:, 3] = -S0 * mu[cB] / sigma[cB] + S0 * bias_of(5)

    return wblk, bblk, affv


def pack_x(x, perm):
    """perm + space-to-depth layout reorg -> [B, 2, 96, 1024] fp32."""
    x = np.asarray(x, np.float32)
    perm = np.asarray(perm)
    B = x.shape[0]
    v = x.reshape(B, 3, 256 * 256)
    v = np.take_along_axis(v, perm[None].astype(np.int64), axis=2)
    v = v.reshape(B, 3, 32, 8, 32, 8)
    v = v.transpose(0, 1, 3, 5, 2, 4).reshape(B, 192, HW)
    return np.ascontiguousarray(v.reshape(B, 2, 96, HW))


# --------------------------------------------------------------------------
# bass program
# --------------------------------------------------------------------------

def build_bass(nb):
    import concourse.bacc as bacc
    import concourse.mybir as mybir
    import concourse.tile as tile
    from concourse.mybir import dt, ActivationFunctionType as AF, AluOpType as ALU

    nc = bacc.Bacc("TRN2", target_bir_lowering=False, debug=False,
                   enable_asserts=False, num_devices=8)

    xin = nc.dram_tensor("xin", [2, 96, HW], dt.float32, kind="ExternalInput").ap()
    affv_d = nc.dram_tensor("affv", [96, 4], dt.float32, kind="ExternalInput").ap()
    wblk_d = nc.dram_tensor("wblk", [nb, 128, WCOLS], dt.float16, kind="ExternalInput").ap()
    bblk_d = nc.dram_tensor("bblk", [128, 6 * nb], dt.float32, kind="ExternalInput").ap()
    yout = nc.dram_tensor("yout", [2, 96, HW], dt.float32, kind="ExternalOutput").ap()

    PADF = 34 * 34  # padded fp16 plane

    with tile.TileContext(nc) as tc:
        with (
            tc.tile_pool(name="wpool", bufs=3) as wpool,
            tc.tile_pool(name="fpool", bufs=1) as fpool,
            tc.tile_pool(name="spool", bufs=4) as spool,
            tc.tile_pool(name="iopool", bufs=1) as iopool,
            tc.tile_pool(name="pspool", bufs=6, space="PSUM") as pspool,
        ):
            # PE warmup: input/weight DMAs leave the PE cold for ~5us at start
            # (p-state + HAM throttle need ~3us of continuous busy). Dummy
            # matmuls on scratch tiles bring it to full clock just as the real
            # stream's dependencies land.
            warm_w = fpool.tile([128, 128], dt.float16, name="warmw", tag="warmw")
            nc.gpsimd.memset(warm_w, 0.0)
            warm_x = fpool.tile([128, 256], dt.float16, name="warmx", tag="warmx")
            nc.gpsimd.memset(warm_x, 0.0)
            warm_ps = pspool.tile([128, N_CHUNK], dt.float32, name="warmps", tag="ps")
            for i in range(24):
                nc.tensor.matmul(warm_ps[:, 0:256], warm_w, warm_x,
                                 start=(i == 0), stop=(i == 23))

            # persistent zero-padded fp16 planes: [pbB, t1, t2, h1, h2].
            # Only the 1-px border must be zero (interior is fully rewritten
            # every use); border memsets go on the otherwise-idle GpSimd so
            # the DVE is free for the startup affine.
            pads = []
            for i in range(5):
                pb = fpool.tile([128, PADF], dt.float16, name=f"pad{i}", tag=f"pad{i}")
                v = pb.rearrange("c (h w) -> c h w", h=34)
                nc.gpsimd.memset(v[:, 0:1, :], 0.0)
                nc.gpsimd.memset(v[:, 33:34, :], 0.0)
                nc.gpsimd.memset(v[:, :, 0:1], 0.0)
                nc.gpsimd.memset(v[:, :, 33:34], 0.0)
                pads.append(pb)

            def pv(i, p0=128):
                return pads[i].rearrange("c (h w) -> c h w", h=34)[0:p0]

            # startup critical path: first matmul needs affine(xB) -> fp16 pad
            # and the conv1 weight panel; everything else queues behind.
            afft = iopool.tile([96, 4], dt.float32, name="afft")
            nc.sync.dma_start(out=afft, in_=affv_d)
            # xB in row-halves: the first conv1 chunk only reads pad rows 0-17,
            # so the first matmul can start on the first half
            xB = spool.tile([96, HW], dt.float32, name="xB", tag="state")
            nc.sync.dma_start(out=xB[:, 0:544], in_=xin[1][:, 0:544])
            wt0 = wpool.tile([128, WCOLS], dt.float16, name="wt", tag="wt")
            nc.sync.dma_start(out=wt0[:, 0:1152], in_=wblk_d[0][:, 0:1152])
            nc.sync.dma_start(out=xB[:, 544:HW], in_=xin[1][:, 544:HW])
            xA = spool.tile([96, HW], dt.float32, name="xA", tag="state")
            nc.sync.dma_start(out=xA, in_=xin[0])
            nc.sync.dma_start(out=wt0[:, 1152:WCOLS], in_=wblk_d[0][:, 1152:WCOLS])
            # all 32 blocks' biases in one small DMA
            btall = iopool.tile([128, 6 * nb], dt.float32, name="btall")
            nc.sync.dma_start(out=btall, in_=bblk_d)

            # fp16 affine(xB) straight into pad0 interior on ACT — this is the
            # startup critical path (first conv1 matmul waits only on this);
            # split at row 17 to match the xB DMA halves
            xBv = xB.rearrange("c (h w) -> c h w", h=32)
            nc.scalar.activation(pv(0, 96)[:, 1:18, 1:33], xBv[:, 0:17, :],
                                 AF.Identity, bias=afft[:, 3:4], scale=afft[:, 2:3])
            nc.scalar.activation(pv(0, 96)[:, 18:33, 1:33], xBv[:, 17:32, :],
                                 AF.Identity, bias=afft[:, 3:4], scale=afft[:, 2:3])
            B32 = spool.tile([96, HW], dt.float32, name="B32", tag="state")
            nc.vector.tensor_scalar(B32, xB, afft[:, 2:3], afft[:, 3:4], ALU.mult, ALU.add)
            A32 = spool.tile([96, HW], dt.float32, name="A32", tag="state")
            nc.vector.tensor_scalar(A32, xA, afft[:, 0:1], afft[:, 1:2], ALU.mult, ALU.add)

            def conv_mms(ps_pair, wt, wcol, oc, ic, rhs_pad_idx):
                """9-tap conv matmuls; chunk n goes to its own 1-bank psum tile.

                Edge-row taps are trimmed: output rows whose source is the
                zero-pad row contribute w*0, so those PE rows are skipped.
                Column trims are NOT done — 31-wide psum writes are unaligned
                and cost more in per-mm overhead than the rows they save.
                Tap (dy=1,dx=1) goes first, untrimmed, with start=True to
                initialize the full psum region."""
                src = pv(rhs_pad_idx, ic)
                order = (4, 0, 1, 2, 3, 5, 6, 7, 8)
                for n, ps in enumerate(ps_pair):
                    psv = ps.rearrange("c (h w) -> c h w", h=16)
                    y0 = 16 * n
                    for i, t in enumerate(order):
                        dy, dx = divmod(t, 3)
                        r0 = 1 if (n == 0 and dy == 0) else 0
                        r1 = 15 if (n == 1 and dy == 2) else 16
                        c0 = 0
                        c1 = 32
                        nc.tensor.matmul(
                            psv[:, r0:r1, c0:c1],
                            wt[0:ic, wcol + t * oc: wcol + (t + 1) * oc],
                            src[:, y0 + dy + r0:y0 + dy + r1, dx + c0:dx + c1],
                            start=(i == 0), stop=(i == 8),
                        )

            def relu_to_pad(ps_pair, oc, pad_idx, bias_ap):
                """relu(psum + bias) -> fp16 pad interior. 3-way split: ACT takes
                rows 0-15 (early, overlaps chunk1 MMs) + the row-16 sliver so the
                next conv's chunk0 unblocks fast; DVE takes rows 17-31 in
                parallel."""
                va = ps_pair[0].rearrange("c (h w) -> c h w", h=16)
                vb = ps_pair[1].rearrange("c (h w) -> c h w", h=16)
                dst = pv(pad_idx, oc)
                nc.scalar.activation(dst[:, 1:17, 1:33], va, AF.Relu, bias=bias_ap)
                nc.scalar.activation(dst[:, 17:18, 1:33], vb[:, 0:1, :], AF.Relu, bias=bias_ap)
                nc.vector.tensor_scalar(dst[:, 18:33, 1:33], vb[:, 1:16, :],
                                        bias_ap, 0.0, ALU.add, ALU.max)

            PB, T1, T2, H1, H2 = 0, 1, 2, 3, 4

            for blk in range(nb):
                if blk == 0:
                    wt = wt0
                else:
                    wt = wpool.tile([128, WCOLS], dt.float16, name="wt", tag="wt")
                    nc.sync.dma_start(out=wt, in_=wblk_d[blk])
                bt = btall[:, 6 * blk:6 * blk + 6]

                src_idx = PB
                for r in range(2):
                    base = 3168 * r
                    ps1 = [pspool.tile([128, N_CHUNK], dt.float32, name=f"ps1{n}", tag="ps")
                           for n in range(2)]
                    conv_mms(ps1, wt, base, 128, 96, src_idx)
                    relu_to_pad(ps1, 128, T1, bt[0:128, 2 * r:2 * r + 1])

                    ps2 = [pspool.tile([128, N_CHUNK], dt.float32, name=f"ps2{n}", tag="ps")
                           for n in range(2)]
                    conv_mms(ps2, wt, base + 1152, 128, 128, T1)
                    relu_to_pad(ps2, 128, T2, bt[0:128, 2 * r + 1:2 * r + 2])

                    ps3 = [pspool.tile([96, N_CHUNK], dt.float32, name=f"ps3{n}", tag="ps")
                           for n in range(2)]
                    conv_mms(ps3, wt, base + 2304, 96, 128, T2)

                    # Feistel add fused straight into the fp16 padded write
                    # (the f32 copy of the sum is never read again).
                    hidx = H1 if r == 0 else H2
                    dst = pv(hidx, 96)
                    v3a = ps3[0].rearrange("c (h w) -> c h w", h=16)
                    v3b = ps3[1].rearrange("c (h w) -> c h w", h=16)
                    A32v = A32.rearrange("c (h w) -> c h w", h=32)
                    nc.vector.tensor_add(dst[:, 1:17, 1:33], v3a, A32v[:, 0:16, :])
                    nc.vector.tensor_add(dst[:, 17:18, 1:33], v3b[:, 0:1, :], A32v[:, 16:17, :])
                    nc.vector.tensor_add(dst[:, 18:33, 1:33], v3b[:, 1:16, :], A32v[:, 17:32, :])
                    A32, B32 = B32, None
                    src_idx = hidx

                # mix: B half first (its fp16 copy gates the next block's conv1)
                def mix_mms(ocol):
                    pair = [pspool.tile([96, N_CHUNK], dt.float32, name=f"psm{n}", tag="ps")
                            for n in range(2)]
                    for n, ps in enumerate(pair):
                        psv = ps.rearrange("c (h w) -> c h w", h=16)
                        y0 = 16 * n
                        nc.tensor.matmul(
                            psv, wt[0:96, MIXBASE + ocol:MIXBASE + ocol + 96],
                            pv(H1, 96)[:, 1 + y0:17 + y0, 1:33], start=True, stop=False)
                        nc.tensor.matmul(
                            psv, wt[0:96, MIXBASE + 192 + ocol:MIXBASE + 192 + ocol + 96],
                            pv(H2, 96)[:, 1 + y0:17 + y0, 1:33], start=False, stop=True)
                    return pair

                if blk + 1 < nb:
                    psB = mix_mms(96)
                    gamma = 1.0 / G
                    # fp16 copy for next conv1 straight off PSUM on ACT
                    dstB = pv(PB, 96)
                    for n, ps in enumerate(psB):
                        psv = ps.rearrange("c (h w) -> c h w", h=16)
                        nc.scalar.activation(dstB[:, 1 + 16 * n:17 + 16 * n, 1:33], psv,
                                             AF.Identity, bias=bt[0:96, 5:6], scale=gamma)
                    B32 = spool.tile([96, HW], dt.float32, name="mB", tag="state")
                    Bv = B32.rearrange("c (h w) -> c h w", h=32)
                    for n, ps in enumerate(psB):
                        nc.vector.tensor_scalar(Bv[:, 16 * n:16 * n + 16, :],
                                                ps.rearrange("c (h w) -> c h w", h=16),
                                                gamma, bt[0:96, 5:6], ALU.mult, ALU.add)
                    psA = mix_mms(0)
                    A32 = spool.tile([96, HW], dt.float32, name="mA", tag="state")
                    Av = A32.rearrange("c (h w) -> c h w", h=32)
                    for n, ps in enumerate(psA):
                        nc.vector.tensor_scalar(Av[:, 16 * n:16 * n + 16, :],
                                                ps.rearrange("c (h w) -> c h w", h=16),
                                                gamma, bt[0:96, 4:5], ALU.mult, ALU.add)
                else:
                    # final block: stream each 512-wide chunk to HBM as soon as
                    # its epilogue lands, overlapping the drain with compute
                    gamma = float(G ** (nb - 1) / S0)
                    for half, yslot in ((0, 0), (96, 1)):
                        pair = mix_mms(half)
                        ot = spool.tile([96, HW], dt.float32, name="o", tag="state")
                        ov = ot.rearrange("c (h w) -> c h w", h=32)
                        yv = yout[yslot].rearrange("c (h w) -> c h w", h=32)
                        for n, ps in enumerate(pair):
                            nc.vector.tensor_scalar(ov[:, 16 * n:16 * n + 16, :],
                                                    ps.rearrange("c (h w) -> c h w", h=16),
                                                    gamma, None, ALU.mult)
                            nc.sync.dma_start(out=yv[:, 16 * n:16 * n + 16, :],
                                              in_=ov[:, 16 * n:16 * n + 16, :])

    nc.compile()
    return nc


# --------------------------------------------------------------------------
# entry point
# --------------------------------------------------------------------------

_last_results = None


def kernel(x, mu, sigma, w_flat, b_flat, m, perm, ops):
    global _last_results
    from concourse.bass_utils import run_bass_kernel_spmd

    nb = _nblocks()
    x = np.asarray(x)
    B = x.shape[0]
    n_cores = 8
    assert B == n_cores, f"expected batch 8, got {B}"

    wblk, bblk, affv = pack_inputs(x, mu, sigma, w_flat, b_flat, m)
    xs = pack_x(x, perm)

    nc = build_bass(nb)

    in_maps = []
    for b in range(B):
        in_maps.append({
            "xin": np.ascontiguousarray(xs[b]),
            "affv": affv,
            "wblk": wblk,
            "bblk": bblk,
        })

    trace = bool(int(os.environ.get("BASS_NN_TRACE", "0")))
    res = run_bass_kernel_spmd(nc, in_maps, core_ids=list(range(n_cores)),
                               trace=trace)
    _last_results = res

    out = np.empty((B, 192, 32, 32), np.float32)
    for b in range(B):
        y = res.results[b]["yout"]  # [2, 96, HW]
        out[b] = y.reshape(192, 32, 32)
    return out

